# revision 1
# baseline (speedup 1.0000x reference)
"""Trainium2 Bass kernel for nn_Attention_Critic (gnn_message_passing).

Strategy: data-parallel over the batch (8 cores x 4096), feature-major
layout on chip ([features, batch]), BatchNorm folded into first-layer
weights (stats via one tiny cross-core AllReduce), attention-weight
products folded on host (sel@key^T and -ckey@csel^T for logit diffs).

Attention is computed in sigmoid/difference form: softmax over 2 (resp.
3) slots needs only logit DIFFERENCES (PE column-reduce matmuls on
encoder-difference products), the last weight is 1/(1+e^dl) (a single
reciprocal, no normalizing multiply for the 2-way groups), and the
weighted sum is v_base + w*(v_other - v_base), whose base-value term
folds into the next matmul's PSUM accumulation (no ov adds).

Per-sample softmax weights are replicated across partitions on the PE:
matmul(lhsT=w_column.broadcast_to((128,128)), rhs=identity) writes
out[f,b] = w[b] straight into PSUM (no DMA broadcast, no transpose
round-trip); the value-weighting multiply reads PSUM directly.

Engines: PE for all matmuls/broadcasts, Act for activations (incl.
LeakyReLU with folded bias), DVE for PSUM-reading multiplies + one
2-op LeakyReLU, Pool(GpSimd, SBUF-only) for value differences and
small softmax arithmetic. bf16 matmuls with fp32 PSUM/stats. PSUM is
tag-partitioned into 8 banks (T0 2, T1 3, T2 2, T4 1) with ping-pong
tag assignment so broadcast->multiply->merge chains overlap.
"""
import os
import sys

sys.path.insert(0, "/opt/trn_rl_repo")

import numpy as np
import ml_dtypes
from contextlib import ExitStack

import concourse.bass as bass
import concourse.tile as tile
from concourse import bacc, mybir
from concourse.bass_utils import run_bass_kernel_spmd
from concourse.masks import make_identity

# Pin every activation to the natural_log_exp_and_others table set (covers
# Exp/Ln/Prelu/Identity/Square/Copy) so the whole kernel needs exactly one
# ACT_TABLE_LOAD instead of thrashing between per-function sets.
_ORIG_GAT = bacc.get_activation_tables


def _pinned_tables(arch):
    t = _ORIG_GAT(arch)
    return {k: (v if k == "natural_log_exp_and_others" else set())
            for k, v in t.items()}


bacc.get_activation_tables = _pinned_tables

NA, B, H = 3, 32768, 128
EPS = 1e-5
NCORES = 8
BS = B // NCORES          # 4096 per core
NT = 512                  # batch tile
ITERS = BS // NT          # 8
NPAIR = ITERS // 2        # 4 iteration pairs
SCALE = 1.0 / np.sqrt(H)

bf16 = mybir.dt.bfloat16
f32 = mybir.dt.float32

BLOCKS = [("en", 0, 6), ("oa0", 7, 4), ("oa1", 12, 4), ("g0", 17, 2),
          ("g1", 20, 2), ("g2", 23, 2), ("senc", 26, 20)]
BLOCK_STAT = {"en": 0, "oa0": 6, "oa1": 10, "g0": 14, "g1": 16, "g2": 18,
              "senc": 0}
BIGW = (["wsk0", "wsk1", "aval0", "aval1", "mcrit", "mcritn", "cvalw"]
        + [f"m_en{n}" for n in range(NA)] + [f"m_ov0{n}" for n in range(NA)]
        + [f"m_ov1{n}" for n in range(NA)] + [f"cw1a{n}" for n in range(NA)]
        + [f"cw1b{n}" for n in range(NA)])
BIASC = ["avb0", "avb1", "mb0", "mb1", "mb2", "cvb", "cb10", "cb11", "cb12"]

LR = mybir.ActivationFunctionType.Prelu


def _b16(x):
    return np.asarray(x, np.float32).astype(ml_dtypes.bfloat16)


def _prep_ent_blocks(s, a, lo, hi):
    rows = []
    for n in range(NA):
        sn = s[n, lo:hi].T
        an = a[n, lo:hi].T
        ones = np.ones((1, hi - lo), np.float32)
        rows += [sn[0:4], an[0:2], ones]
        rows += [sn[4:8], ones, sn[8:12], ones]
        rows += [sn[12:14], ones, sn[14:16], ones, sn[16:18], ones]
        rows += [sn[0:4], an[0:2], sn[4:18], ones]
    return np.ascontiguousarray(np.concatenate(rows, 0), dtype=np.float32)


def _prep_l1w(inp):
    out = np.zeros((141, 128), np.float32)
    for n in range(NA):
        o = 47 * n
        out[o + 0:o + 6] = inp["en_W"][n]
        out[o + 6] = inp["en_b"][n]
        out[o + 7:o + 11] = inp["oa_W"][n]
        out[o + 11] = inp["oa_b"][n]
        out[o + 12:o + 16] = inp["oa_W"][n]
        out[o + 16] = inp["oa_b"][n]
        out[o + 17:o + 19] = inp["goal_W"][n]
        out[o + 19] = inp["goal_b"][n]
        out[o + 20:o + 22] = inp["goal_W"][n]
        out[o + 22] = inp["goal_b"][n]
        out[o + 23:o + 25] = inp["goal_W"][n]
        out[o + 25] = inp["goal_b"][n]
        out[o + 26:o + 30] = inp["senc_W"][n][0:4]
        out[o + 32:o + 46] = inp["senc_W"][n][4:18]
        out[o + 46] = inp["senc_b"][n]
    return out


def _prep_bigw(inp):
    w = {}
    w["wsk0"] = inp["asel_W"][0] @ inp["akey_W"][0].T
    w["wsk1"] = inp["asel_W"][1] @ inp["akey_W"][1].T
    w["aval0"] = inp["aval_W"][0]
    w["aval1"] = inp["aval_W"][1]
    w["mcrit"] = inp["ckey_W"][0] @ inp["csel_W"][0].T
    w["mcritn"] = -w["mcrit"]
    w["cvalw"] = inp["cval_W"][0]
    for n in range(NA):
        w[f"m_en{n}"] = inp["merge_W"][n, 0:128]
        w[f"m_ov0{n}"] = inp["merge_W"][n, 128:256]
        w[f"m_ov1{n}"] = inp["merge_W"][n, 256:384]
        w[f"cw1a{n}"] = inp["cW1"][n, 0:128]
        w[f"cw1b{n}"] = inp["cW1"][n, 128:256]
    return _b16(np.concatenate([w[k] for k in BIGW], 0))


def _prep_bias(inp):
    cols = [inp["aval_b"][0], inp["aval_b"][1],
            inp["merge_b"][0], inp["merge_b"][1], inp["merge_b"][2],
            inp["cval_b"][0], inp["cb1"][0], inp["cb1"][1], inp["cb1"][2]]
    return np.stack(cols, 1).astype(np.float32)


_NC_CACHE = {}


def _build_nc():
    nc = bacc.Bacc("TRN2", target_bir_lowering=False, debug=False,
                   num_devices=NCORES)
    entd = nc.dram_tensor("entd", [141, BS], f32, kind="ExternalInput")
    l1wd = nc.dram_tensor("l1wd", [141, 128], f32, kind="ExternalInput")
    bigwd = nc.dram_tensor("bigwd", [22 * 128, 128], bf16, kind="ExternalInput")
    cw2d = nc.dram_tensor("cw2d", [NA * 128, 2], bf16, kind="ExternalInput")
    biasd = nc.dram_tensor("biasd", [128, 9], f32, kind="ExternalInput")
    cb2d = nc.dram_tensor("cb2d", [2, NA], f32, kind="ExternalInput")
    outd = nc.dram_tensor("outd", [6, BS], f32, kind="ExternalOutput")

    cc_in = nc.dram_tensor("cc_in", [60, 2], f32)
    cc_out = nc.dram_tensor("cc_out", [60, 2], f32, addr_space="Shared")

    with tile.TileContext(nc) as tc, ExitStack() as ctx:
        wp = ctx.enter_context(tc.tile_pool(name="wp", bufs=1))
        io = ctx.enter_context(tc.tile_pool(name="io", bufs=1))
        wk = ctx.enter_context(tc.tile_pool(name="wk", bufs=2))
        pp = ctx.enter_context(tc.tile_pool(name="pp", bufs=1, space="PSUM"))

        big = {}
        for idx, name in enumerate(BIGW):
            t = wp.tile([128, 128], bf16, name=f"bw_{name}")
            nc.sync.dma_start(t[:], bigwd[128 * idx:128 * (idx + 1), :])
            big[name] = t
        cw2 = []
        for n in range(NA):
            t = wp.tile([128, 2], bf16, name=f"cw2_{n}")
            nc.sync.dma_start(t[:], cw2d[128 * n:128 * (n + 1), :])
            cw2.append(t)
        biast = wp.tile([128, 9], f32)
        nc.sync.dma_start(biast[:], biasd[:, :])
        bcol = {name: biast[:, i:i + 1] for i, name in enumerate(BIASC)}
        cb2t = wp.tile([2, NA], f32)
        nc.sync.dma_start(cb2t[:], cb2d[:, :])
        onesb = wp.tile([128, 1], bf16)
        nc.vector.memset(onesb[:], 1.0)
        ident = wp.tile([128, 128], bf16)
        make_identity(nc, ident[:])
        zbias = wp.tile([128, 1], f32)
        nc.vector.memset(zbias[:], 0.0)

        ebC = {}
        for n in range(NA):
            o = 47 * n
            t = io.tile([21, BS], bf16, name=f"ebC{n}")
            for q in range(4):
                qs_ = slice(1024 * q, 1024 * (q + 1))
                nc.gpsimd.dma_start(t[:, qs_], entd[o + 26:o + 47, qs_])
            ebC[n] = t

        # ---------- stats ----------
        for n in range(NA):
            sq2 = wp.tile([20, 4], f32, name=f"sq2_{n}")
            for c in range(4):
                sqp = pp.tile([20, 1024], f32, name="sqp", tag="T0")
                nc.scalar.activation(
                    sqp[:], ebC[n][0:20, 1024 * c:1024 * (c + 1)],
                    mybir.ActivationFunctionType.Square,
                    accum_out=sq2[:, c:c + 1])
            sumq = wp.tile([20, 1], f32, name=f"sumq_{n}")
            nc.vector.tensor_reduce(out=sumq[:], in_=sq2[:],
                                    op=mybir.AluOpType.add,
                                    axis=mybir.AxisListType.X)
            sumx = wp.tile([20, 1], f32, name=f"sumx_{n}")
            nc.vector.tensor_reduce(out=sumx[:], in_=ebC[n][0:20, :],
                                    op=mybir.AluOpType.add,
                                    axis=mybir.AxisListType.X)
            nc.sync.dma_start(cc_in[20 * n:20 * n + 20, 0:1], sumx[:])
            nc.sync.dma_start(cc_in[20 * n:20 * n + 20, 1:2], sumq[:])
        nc.gpsimd.collective_compute(
            "AllReduce", mybir.AluOpType.add,
            replica_groups=[list(range(NCORES))],
            ins=[cc_in[:, :]], outs=[cc_out[:, :]])
        gst = wp.tile([60, 2], f32)
        nc.sync.dma_start(gst[:], cc_out[:, :])
        mean = wp.tile([60, 1], f32)
        nc.vector.tensor_scalar_mul(mean[:], gst[:, 0:1], 1.0 / B)
        ex2 = wp.tile([60, 1], f32)
        nc.vector.tensor_scalar_mul(ex2[:], gst[:, 1:2], 1.0 / B)
        m2 = wp.tile([60, 1], f32)
        nc.vector.tensor_mul(m2[:], mean[:], mean[:])
        var = wp.tile([60, 1], f32)
        nc.vector.tensor_sub(var[:], ex2[:], m2[:])
        epst = wp.tile([60, 1], f32)
        nc.vector.memset(epst[:], EPS)
        lnv = wp.tile([60, 1], f32)
        nc.scalar.activation(lnv[:], var[:], mybir.ActivationFunctionType.Ln,
                             bias=epst[:])
        std = wp.tile([60, 1], f32)
        nc.scalar.activation(std[:], lnv[:], mybir.ActivationFunctionType.Exp,
                             scale=0.5)
        rstd = wp.tile([60, 1], f32)
        nc.vector.reciprocal(rstd[:], std[:])
        meanb = wp.tile([60, 1], bf16)
        nc.vector.tensor_copy(meanb[:], mean[:])

        # ---------- fold first-layer weights ----------
        # Groups: A rows en(0:7)|oa0(32:37), B oa1(0:5)|g0(32:35)|senc(64:85),
        # C g1(0:3)|g2(32:35) -- row layout inside each group's sbuf tile.
        GRP = {"en": ("A", 0, 6), "oa0": ("A", 32, 4), "oa1": ("A", 64, 4),
               "senc": ("B", 0, 20),
               "g1": ("C", 0, 2), "g2": ("C", 32, 2), "g0": ("C", 64, 2)}
        GSIZE = {"A": 69, "B": 21, "C": 67}
        lwg, blkg, rsbg, mbbg = {}, {}, {}, {}
        for n in range(NA):
            for gname in "ABC":
                gsz = GSIZE[gname]
                lwg[(n, gname)] = wp.tile([gsz, 128], f32, name=f"lw{n}{gname}")
                blkg[(n, gname)] = wp.tile([gsz, 128], bf16,
                                           name=f"blk{n}{gname}")
                rsbg[(n, gname)] = wp.tile([gsz, 1], f32, name=f"rsb{n}{gname}")
                mbbg[(n, gname)] = wp.tile([gsz, 1], bf16,
                                           name=f"mbb{n}{gname}")
        blk = {}
        for n in range(NA):
            o = 47 * n
            for bname, st, K in BLOCKS:
                gname, base, _ = GRP[bname]
                so = 20 * n + BLOCK_STAT[bname]
                lw = lwg[(n, gname)]
                bw = blkg[(n, gname)]
                rsb = rsbg[(n, gname)]
                mbb = mbbg[(n, gname)]
                nc.scalar.dma_start(lw[base:base + K, :],
                                    l1wd[o + st:o + st + K, :])
                braw = wk.tile([1, 128], f32, name="brawtmp", bufs=3)
                nc.scalar.dma_start(braw[:],
                                    l1wd[o + st + K:o + st + K + 1, :])
                nc.sync.dma_start(rsb[base:base + K, :], rstd[so:so + K, :])
                nc.sync.dma_start(mbb[base:base + K, :], meanb[so:so + K, :])
                nc.vector.tensor_scalar_mul(bw[base:base + K, :],
                                            lw[base:base + K, :],
                                            rsb[base:base + K, :])
                pb = pp.tile([1, 128], f32, name="pbias", tag="T4")
                nc.tensor.matmul(pb[:], mbb[base:base + K, :],
                                 bw[base:base + K, :], start=True, stop=True)
                brow = wk.tile([1, 128], bf16, name="browtmp", bufs=3)
                nc.vector.tensor_sub(brow[:], braw[:], pb[:])
                nc.sync.dma_start(bw[base + K:base + K + 1, :], brow[:])
                blk[(n, bname)] = bw[base:base + K + 1, :]

        # ---------- main loop: iteration PAIRS ----------
        # PSUM tags (8 banks):
        #  T0 [128,1024] 2 banks: l1pA(en|oa0) / l1pC(g1|g2) / v0p / wb x3
        #  T1 [128,1536] 3 banks: l1pB(oa1|g0|senc) / v1p / kdp / cvp / cwb x3
        #  T2 [128,1024] 2 banks: skp / mp2 / hp2
        #  T4 [<=512c]   1 bank: lg / wT / clg / cwT / qp
        # Softmax over 2 (resp. 3) slots is computed from logit DIFFERENCES:
        # w_last = 1/(1+e^dl) etc., so only 1 (resp. 2) weight rows need the
        # transpose+broadcast, and the base value folds into the PSUM
        # accumulation of the next matmul (no ov adds).
        for ip in range(NPAIR):
            psl = slice(ip * 2 * NT, (ip + 1) * 2 * NT)
            sa = {}
            se_t = {}
            en_t_t = {}
            vals0_t = {}
            vals1_t = {}
            wbm_t = {}
            ebg = {}
            for n in range(NA):
                o = 47 * n
                ebU = wk.tile([69, 2 * NT], bf16, name=f"ebU{n}", bufs=2)
                ebV = wk.tile([67, 2 * NT], bf16, name=f"ebV{n}", bufs=2)
                tmap = {"en": ebU, "oa0": ebU, "oa1": ebU, "g0": ebV,
                        "g1": ebV, "g2": ebV}
                for bname, st, K in BLOCKS:
                    if bname == "senc":
                        continue
                    g, base, _ = GRP[bname]
                    t = tmap[bname]
                    nc.gpsimd.dma_start(t[base:base + K + 1, :],
                                        entd[o + st:o + st + K + 1, psl])
                    ebg[(n, bname)] = t[base:base + K + 1, :]

            # ---- phase 1: encoders + values + logit differences (per n) ----
            for n in range(NA):
                lg = pp.tile([128, 24], f32, name="lg", tag="T4")
                prs_t = {}
                vals0 = wk.tile([128, 2048], bf16, name="vals0", bufs=3)
                vals1 = wk.tile([128, 3072], bf16, name="vals1", bufs=3)
                for h in range(2):
                    sl = slice((2 * ip + h) * NT, (2 * ip + h + 1) * NT)
                    hsl = slice(h * NT, (h + 1) * NT)
                    l1pA = pp.tile([128, 1024], f32, name="l1pA", tag="T0")
                    nc.tensor.matmul(l1pA[:, 0:512], blk[(n, "en")],
                                     ebg[(n, "en")][:, hsl],
                                     start=True, stop=True)
                    nc.tensor.matmul(l1pA[:, 512:1024], blk[(n, "oa0")],
                                     ebg[(n, "oa0")][:, hsl],
                                     start=True, stop=True)
                    l1pB = pp.tile([128, 1536], f32, name="l1pB", tag="T1")
                    nc.tensor.matmul(l1pB[:, 0:512], blk[(n, "oa1")],
                                     ebg[(n, "oa1")][:, hsl],
                                     start=True, stop=True)
                    nc.tensor.matmul(l1pB[:, 512:1024], blk[(n, "g0")],
                                     ebg[(n, "g0")][:, hsl],
                                     start=True, stop=True)
                    nc.tensor.matmul(l1pB[:, 1024:1536], blk[(n, "senc")],
                                     ebC[n][:, sl], start=True, stop=True)
                    en_t = wk.tile([128, 512], bf16, name="en_t", bufs=7)
                    l1r = wk.tile([128, 2560], bf16, name="l1r", bufs=2)
                    se = wk.tile([128, 512], bf16, name="se", bufs=7)
                    nc.scalar.activation(en_t[:], l1pA[:, 0:512], LR,
                                         bias=zbias[:], alpha=0.01)
                    nc.scalar.activation(l1r[:, 0:512], l1pA[:, 512:1024], LR,
                                         bias=zbias[:], alpha=0.01)
                    l1pC = pp.tile([128, 1024], f32, name="l1pC", tag="T0")
                    nc.tensor.matmul(l1pC[:, 0:512], blk[(n, "g1")],
                                     ebg[(n, "g1")][:, hsl],
                                     start=True, stop=True)
                    nc.tensor.matmul(l1pC[:, 512:1024], blk[(n, "g2")],
                                     ebg[(n, "g2")][:, hsl],
                                     start=True, stop=True)
                    btmp = wk.tile([128, 1024], bf16, name="btmp", bufs=2)
                    nc.vector.tensor_scalar_mul(btmp[:], l1pB[:, 0:1024],
                                                0.01)
                    nc.vector.tensor_tensor(out=l1r[:, 512:1536],
                                            in0=btmp[:], in1=l1pB[:, 0:1024],
                                            op=mybir.AluOpType.max)
                    nc.scalar.activation(se[:], l1pB[:, 1024:1536], LR,
                                         bias=zbias[:], alpha=0.01)
                    nc.scalar.activation(l1r[:, 1536:2560], l1pC[:], LR,
                                         bias=zbias[:], alpha=0.01)
                    en_t_t[(h, n)] = en_t
                    se_t[(h, n)] = se
                    skp = pp.tile([128, 1024], f32, name="skp", tag="T2")
                    nc.tensor.matmul(skp[:, 0:512], big["wsk0"][:], en_t[:],
                                     start=True, stop=True)
                    nc.tensor.matmul(skp[:, 512:1024], big["wsk1"][:],
                                     en_t[:], start=True, stop=True)
                    v0p = pp.tile([128, 1024], f32, name="v0p", tag="T0")
                    nc.tensor.matmul(v0p[:, 0:512], big["aval0"][:],
                                     l1r[:, 0:512], start=True, stop=True)
                    nc.tensor.matmul(v0p[:, 512:1024], big["aval0"][:],
                                     l1r[:, 512:1024], start=True, stop=True)
                    nc.scalar.activation(vals0[:, 1024 * h:1024 * (h + 1)],
                                         v0p[:], LR, bias=bcol["avb0"],
                                         alpha=0.01)
                    v1p = pp.tile([128, 1536], f32, name="v1p", tag="T1")
                    nc.tensor.matmul(v1p[:, 0:512], big["aval1"][:],
                                     l1r[:, 1024:1536], start=True, stop=True)
                    nc.tensor.matmul(v1p[:, 512:1024], big["aval1"][:],
                                     l1r[:, 1536:2048], start=True, stop=True)
                    nc.tensor.matmul(v1p[:, 1024:1536], big["aval1"][:],
                                     l1r[:, 2048:2560], start=True, stop=True)
                    nc.scalar.activation(vals1[:, 1536 * h:1536 * (h + 1)],
                                         v1p[:], LR, bias=bcol["avb1"],
                                         alpha=0.01)
                    # logit differences: d01 = l0-l1 (2-way group),
                    # d32 = l3-l2, d42 = l4-l2 (3-way group)
                    prs = []
                    for c, (pa, pb_) in enumerate(((0, 1), (3, 2), (4, 2))):
                        ed = wk.tile([128, 512], bf16, name="ed", bufs=4)
                        deng = nc.gpsimd if c == 0 else nc.vector
                        deng.tensor_tensor(
                            out=ed[:], in0=l1r[:, 512 * pa:512 * (pa + 1)],
                            in1=l1r[:, 512 * pb_:512 * (pb_ + 1)],
                            op=mybir.AluOpType.subtract)
                        sk = skp[:, 0:512] if c == 0 else skp[:, 512:1024]
                        pr = wk.tile([128, 512], bf16, name="pr", bufs=6)
                        nc.vector.tensor_tensor(out=pr[:], in0=ed[:],
                                                in1=sk,
                                                op=mybir.AluOpType.mult)
                        prs.append(pr)
                    prs_t[h] = prs
                for h in range(2):
                    for c in range(3):
                        for t in range(4):
                            col = 8 * c + 4 * h + t
                            nc.tensor.matmul(lg[:, col:col + 1],
                                             prs_t[h][c][:,
                                                         128 * t:128 * (t + 1)],
                                             onesb[:], start=True, stop=True)
                vals0_t[n] = vals0
                vals1_t[n] = vals1
                # sigmoid-form softmax (batch-major, tiny) for this n
                ebm = wk.tile([128, 24], bf16, name="ebm", bufs=3)
                nc.scalar.activation(ebm[:], lg[:],
                                     mybir.ActivationFunctionType.Exp,
                                     scale=SCALE)
                den = wk.tile([128, 16], f32, name="den", bufs=3)
                nc.gpsimd.tensor_scalar_add(den[:, 0:8], ebm[:, 0:8], 1.0)
                nc.vector.scalar_tensor_tensor(
                    out=den[:, 8:16], in0=ebm[:, 8:16], scalar=1.0,
                    in1=ebm[:, 16:24], op0=mybir.AluOpType.add,
                    op1=mybir.AluOpType.add)
                rec = wk.tile([128, 16], f32, name="rec", bufs=3)
                nc.vector.reciprocal(rec[:], den[:])
                wbm = wk.tile([128, 24], bf16, name="wbm", bufs=3)
                nc.gpsimd.tensor_copy(wbm[:, 0:8], rec[:, 0:8])
                nc.gpsimd.tensor_tensor(
                    out=wbm[:, 8:24].rearrange("p (c t) -> p c t", c=2),
                    in0=ebm[:, 8:24].rearrange("p (c t) -> p c t", c=2),
                    in1=rec[:, 8:16].rearrange("p (u t) -> p u t", u=1)
                    .broadcast_to((128, 2, 8)),
                    op=mybir.AluOpType.mult)
                wbm_t[n] = wbm

            # ---- phase 2: weight transpose/broadcast + merge ----
            for n in range(NA):
                v0v = vals0_t[n][:, :].rearrange("p (h q b) -> p h q b",
                                                 h=2, q=2)
                v1v = vals1_t[n][:, :].rearrange("p (h q b) -> p h q b",
                                                 h=2, q=3)
                # value differences (SBUF only, Pool engine)
                dv0 = wk.tile([128, 1024], bf16, name="dv0", bufs=2)
                nc.gpsimd.tensor_tensor(
                    out=dv0[:].rearrange("p (h b) -> p h b", h=2),
                    in0=v0v[:, :, 1, :], in1=v0v[:, :, 0, :],
                    op=mybir.AluOpType.subtract)
                dv3 = wk.tile([128, 1024], bf16, name="dv3", bufs=2)
                nc.gpsimd.tensor_tensor(
                    out=dv3[:].rearrange("p (h b) -> p h b", h=2),
                    in0=v1v[:, :, 1, :], in1=v1v[:, :, 0, :],
                    op=mybir.AluOpType.subtract)
                dv4 = wk.tile([128, 1024], bf16, name="dv4", bufs=2)
                nc.gpsimd.tensor_tensor(
                    out=dv4[:].rearrange("p (h b) -> p h b", h=2),
                    in0=v1v[:, :, 2, :], in1=v1v[:, :, 0, :],
                    op=mybir.AluOpType.subtract)
                wbm = wbm_t[n]
                scs = []
                mp2 = pp.tile([128, 1024], f32, name="mp2", tag="T2")

                def _mp2sc(c):
                    mw = big[f"m_ov0{n}"] if c == 0 else big[f"m_ov1{n}"]
                    for h in range(2):
                        hs = slice(512 * h, 512 * (h + 1))
                        nc.tensor.matmul(mp2[:, hs], mw[:], scs[c][:, hs],
                                         start=False, stop=(c == 2))

                for c, dv in enumerate((dv0, dv3, dv4)):
                    wb = pp.tile([128, 1024], f32, name="wb",
                                 tag=(("T0", "T1", "T0") if n % 2 == 0
                                      else ("T1", "T0", "T1"))[c])
                    for t8 in range(8):
                        col = 8 * c + t8
                        nc.tensor.matmul(wb[:, 128 * t8:128 * (t8 + 1)],
                                         wbm[:, col:col + 1]
                                         .broadcast_to((128, 128)),
                                         ident[:], start=True, stop=True)
                    if c == 0:
                        for h in range(2):
                            hs = slice(512 * h, 512 * (h + 1))
                            nc.tensor.matmul(mp2[:, hs], big[f"m_en{n}"][:],
                                             en_t_t[(h, n)][:], start=True,
                                             stop=False)
                            nc.tensor.matmul(
                                mp2[:, hs], big[f"m_ov0{n}"][:],
                                vals0_t[n][:, 1024 * h:1024 * h + 512],
                                start=False, stop=False)
                            nc.tensor.matmul(
                                mp2[:, hs], big[f"m_ov1{n}"][:],
                                vals1_t[n][:, 1536 * h:1536 * h + 512],
                                start=False, stop=False)
                    sc = wk.tile([128, 1024], bf16, name="sc", bufs=4)
                    for h in range(2):
                        nc.vector.tensor_tensor(
                            out=sc[:, 512 * h:512 * (h + 1)],
                            in0=dv[:, 512 * h:512 * (h + 1)],
                            in1=wb[:, 512 * h:512 * (h + 1)],
                            op=mybir.AluOpType.mult)
                    scs.append(sc)
                    if c == 1:
                        _mp2sc(0)
                _mp2sc(1)
                _mp2sc(2)
                for h in range(2):
                    sa_n = wk.tile([128, 512], bf16, name="sa", bufs=7)
                    nc.scalar.activation(sa_n[:],
                                         mp2[:, 512 * h:512 * (h + 1)], LR,
                                         bias=bcol[f"mb{n}"], alpha=0.01)
                    sa[(h, n)] = sa_n

            # ---- critic (sigmoid form: only 2 neighbours per agent) ----
            # i=0: (j0,j1)=(1,2); i=1: (0,2); i=2: (0,1)
            JP = [(1, 2), (0, 2), (0, 1)]
            cval = wk.tile([128, 3072], bf16, name="cval", bufs=2)
            clg = pp.tile([128, 24], f32, name="clg", tag="T4")
            for h in range(2):
                if h == 0:
                    kdp = pp.tile([128, 1536], f32, name="kdp", tag="T1")
                    kslice = [kdp[:, 512 * i:512 * (i + 1)] for i in range(3)]
                else:
                    kdpa = pp.tile([128, 1024], f32, name="kdpa", tag="T0")
                    kdpb = pp.tile([128, 512], f32, name="kdpb", tag="T2")
                    kslice = [kdpa[:, 0:512], kdpa[:, 512:1024], kdpb[:]]
                for i, (j0, j1) in enumerate(JP):
                    nc.tensor.matmul(kslice[i], big["mcrit"][:],
                                     sa[(h, j0)][:], start=True, stop=False)
                    nc.tensor.matmul(kslice[i], big["mcritn"][:],
                                     sa[(h, j1)][:], start=False, stop=True)
                for i in range(NA):
                    prc = wk.tile([128, 512], bf16, name="prc", bufs=6)
                    nc.vector.tensor_tensor(
                        out=prc[:], in0=se_t[(h, i)][:],
                        in1=kslice[i],
                        op=mybir.AluOpType.mult)
                    for t in range(4):
                        col = 8 * i + 4 * h + t
                        nc.tensor.matmul(
                            clg[:, col:col + 1],
                            prc[:, 128 * t:128 * (t + 1)],
                            onesb[:], start=True, stop=True)
            cebm = wk.tile([128, 24], bf16, name="cebm")
            nc.scalar.activation(cebm[:], clg[:],
                                 mybir.ActivationFunctionType.Exp, scale=SCALE)
            cden = wk.tile([128, 24], f32, name="cden")
            nc.gpsimd.tensor_scalar_add(cden[:], cebm[:], 1.0)
            crec = wk.tile([128, 24], f32, name="crec")
            nc.vector.reciprocal(crec[:], cden[:])
            cwbm = wk.tile([128, 24], bf16, name="cwbm")
            nc.gpsimd.tensor_copy(cwbm[:], crec[:])
            for h in range(2):
                cvp = pp.tile([128, 1536], f32, name="cvp", tag="T1")
                for j in range(NA):
                    nc.tensor.matmul(cvp[:, 512 * j:512 * (j + 1)],
                                     big["cvalw"][:], sa[(h, j)][:],
                                     start=True, stop=True)
                nc.scalar.activation(cval[:, 1536 * h:1536 * (h + 1)],
                                     cvp[:], LR, bias=bcol["cvb"], alpha=0.01)
            cvv = cval[:, :].rearrange("p (h j b) -> p h j b", h=2, j=3)
            dcvs = []
            for i in range(NA):
                j0, j1 = JP[i]
                dcv = wk.tile([128, 1024], bf16, name="dcv", bufs=4)
                nc.gpsimd.tensor_tensor(
                    out=dcv[:].rearrange("p (h b) -> p h b", h=2),
                    in0=cvv[:, :, j1, :], in1=cvv[:, :, j0, :],
                    op=mybir.AluOpType.subtract)
                dcvs.append(dcv)
            for i in range(NA):
                j0, j1 = JP[i]
                dcv = dcvs[i]
                cwb = pp.tile([128, 1024], f32, name="cwb",
                              tag=("T1", "T0", "T1")[i])
                for t8 in range(8):
                    col = 8 * i + t8
                    nc.tensor.matmul(cwb[:, 128 * t8:128 * (t8 + 1)],
                                     cwbm[:, col:col + 1]
                                     .broadcast_to((128, 128)),
                                     ident[:], start=True, stop=True)
                hp2 = pp.tile([128, 1024], f32, name="hp2", tag="T2")
                for h in range(2):
                    hs = slice(512 * h, 512 * (h + 1))
                    nc.tensor.matmul(hp2[:, hs], big[f"cw1a{i}"][:],
                                     se_t[(h, i)][:], start=True, stop=False)
                    nc.tensor.matmul(hp2[:, hs], big[f"cw1b{i}"][:],
                                     cval[:, 1536 * h + 512 * j0:
                                          1536 * h + 512 * (j0 + 1)],
                                     start=False, stop=False)
                csc = wk.tile([128, 1024], bf16, name="csc", bufs=3)
                for h in range(2):
                    nc.vector.tensor_tensor(
                        out=csc[:, 512 * h:512 * (h + 1)],
                        in0=dcv[:, 512 * h:512 * (h + 1)],
                        in1=cwb[:, 512 * h:512 * (h + 1)],
                        op=mybir.AluOpType.mult)
                for h in range(2):
                    hs = slice(512 * h, 512 * (h + 1))
                    nc.tensor.matmul(hp2[:, hs], big[f"cw1b{i}"][:],
                                     csc[:, hs], start=False, stop=True)
                for h in range(2):
                    it = 2 * ip + h
                    sl = slice(it * NT, (it + 1) * NT)
                    h_ = wk.tile([128, 512], bf16, name="h", bufs=3)
                    nc.scalar.activation(h_[:],
                                         hp2[:, 512 * h:512 * (h + 1)], LR,
                                         bias=bcol[f"cb1{i}"], alpha=0.01)
                    qp = pp.tile([2, 512], f32, name="qp", tag="T4")
                    nc.tensor.matmul(qp[:], cw2[i][:], h_[:], start=True,
                                     stop=True)
                    qs = wk.tile([2, 512], f32, name="qs", bufs=3)
                    nc.scalar.activation(qs[:], qp[:],
                                         mybir.ActivationFunctionType.Identity,
                                         bias=cb2t[:, i:i + 1])
                    nc.sync.dma_start(outd[2 * i:2 * i + 2, sl], qs[:])

    nc.compile()
    return nc


def _get_nc():
    if "nc" not in _NC_CACHE:
        _NC_CACHE["nc"] = _build_nc()
    return _NC_CACHE["nc"]


def kernel(s, a, en_W, en_b, oa_W, oa_b, goal_W, goal_b, akey_W, asel_W,
           aval_W, aval_b, merge_W, merge_b, senc_W, senc_b, ckey_W,
           csel_W, cval_W, cval_b, cW1, cb1, cW2, cb2):
    inp = dict(s=s, a=a, en_W=en_W, en_b=en_b, oa_W=oa_W, oa_b=oa_b,
               goal_W=goal_W, goal_b=goal_b, akey_W=akey_W, asel_W=asel_W,
               aval_W=aval_W, aval_b=aval_b, merge_W=merge_W, merge_b=merge_b,
               senc_W=senc_W, senc_b=senc_b, ckey_W=ckey_W, csel_W=csel_W,
               cval_W=cval_W, cval_b=cval_b, cW1=cW1, cb1=cb1, cW2=cW2,
               cb2=cb2)
    inp = {k: np.asarray(v, np.float32) for k, v in inp.items()}
    s_, a_ = inp["s"], inp["a"]

    l1w = _prep_l1w(inp)
    bigw = _prep_bigw(inp)
    cw2 = _b16(np.concatenate([inp["cW2"][n] for n in range(NA)], 0))
    biasc = _prep_bias(inp)
    cb2c = inp["cb2"].T.copy()

    in_maps = []
    for c in range(NCORES):
        ent = _prep_ent_blocks(s_, a_, c * BS, (c + 1) * BS)
        in_maps.append({"entd": ent, "l1wd": l1w, "bigwd": bigw,
                        "cw2d": cw2, "biasd": biasc, "cb2d": cb2c})

    nc = _get_nc()
    trace = os.environ.get("BASS_KERNEL_TRACE") == "1"
    res = run_bass_kernel_spmd(nc, in_maps, core_ids=list(range(NCORES)),
                               trace=trace)
    if trace:
        kernel.last_exec_time_ns = res.exec_time_ns
        kernel.last_results = res

    qfull = np.concatenate([res.results[c]["outd"] for c in range(NCORES)], 1)
    return np.ascontiguousarray(
        np.transpose(qfull.reshape(NA, 2, B), (0, 2, 1))).astype(np.float32)

